# revision 2
# baseline (speedup 1.0000x reference)
"""EnergyHead kernel for Trainium2 (8 NeuronCores, batch-parallel, fp8 DoubleRow).

Computes, per batch element:
    xH = x @ W_H.T
    scores = x @ xH.T  (strict lower-triangular causal mask, diag excluded)
    wei = softmax(scores); fully-masked row 0 zeroed
    out = -(wei @ xH)

Sharding: data-parallel over B=8 across 8 cores. All three matmul phases run
in fp8e4m3 DoubleRow mode (K=256 per instruction, 0.5 cycles/row) with
residual-split operands for precision: every tensor a is represented as
a1 + a2 where a1 = e4m3(a), a2 = e4m3(a - a1), and products use the 3-term
expansion a1b1 + a1b2 + a2b1 (the dropped a2b2 term is O(2^-8) relative).
W is pre-scaled by 16 on the host so all fp8 magnitudes stay well under the
e4m3 max (240); the 1/16 is folded into the softmax bias/scale and the final
recip multiply. End-to-end rel err vs fp32 reference ~3e-3.
"""
import sys
import os
import functools

sys.path.insert(0, "/opt/trn_rl_repo")
import numpy as np
import ml_dtypes

# experiment flags (read once at import)
CFG_SPLIT_WEIT = int(os.environ.get("K_SPLIT_WEIT", "2"))  # 1 | 2
CFG_DEPTH = int(os.environ.get("K_DEPTH", "3"))    # scores prefetch depth
CFG_WARM = int(os.environ.get("K_WARM", "12"))
CFG_TAIL = int(os.environ.get("K_TAIL", "1"))
CFG_Y1 = os.environ.get("K_Y1", "act")             # act | dve
CFG_W1 = os.environ.get("K_W1", "act")             # act | dve
CFG_OSB = os.environ.get("K_OSB", "split")         # act | dve | split

B, T, C = 8, 2048, 1024
NCORES = 8
P = 128                      # partition dim
QT = T // P                  # 16 q-tiles
CCH = C // P                 # 8 contraction 128-chunks
DRC = C // 256               # 4 DoubleRow 256-chunks
TB = T // 512                # 4 t-blocks
CB = C // 512                # 2 column blocks of the output
NEG_BIG = -1e30
WSCALE = 16.0
F8 = ml_dtypes.float8_e4m3


def _sblocks(i: int) -> list[int]:
    """Split S=128*(i+1) score columns into <=512 matmul N-blocks."""
    S = P * (i + 1)
    k4, rem = divmod(S, 512)
    return [512] * k4 + ([rem] if rem else [])


@functools.lru_cache(maxsize=4)
def _build(reps: int = 1):
    import concourse.bacc as bacc
    import concourse.tile as tile
    from concourse import mybir

    f32 = mybir.dt.float32
    fp8 = mybir.dt.float8e4
    u16 = mybir.dt.uint16
    bf16 = mybir.dt.bfloat16
    X = mybir.AxisListType.X
    Exp = mybir.ActivationFunctionType.Exp
    Copy = mybir.ActivationFunctionType.Copy
    DR = mybir.MatmulPerfMode.DoubleRow

    nc = bacc.Bacc("TRN2", target_bir_lowering=False, debug=False,
                   enable_asserts=False, num_devices=NCORES)

    # host layouts:
    #   x1/x2: [p, tb, j, u] = x.T[128j+p, 512tb+u]  (fp8 value + residual)
    #   w1/w2: [p, j, d]     = 16*W.T[128j+p, d]
    x1_d = nc.dram_tensor("x1", [P, TB, CCH, 512], fp8, kind="ExternalInput").ap()
    x2_d = nc.dram_tensor("x2", [P, TB, CCH, 512], fp8, kind="ExternalInput").ap()
    w1_d = nc.dram_tensor("w1", [P, CCH, C], fp8, kind="ExternalInput").ap()
    w2_d = nc.dram_tensor("w2", [P, CCH, C], fp8, kind="ExternalInput").ap()
    out_d = nc.dram_tensor("out", [T, C], f32, kind="ExternalOutput").ap()

    with tile.TileContext(nc) as tc:
      for _rep in range(reps):
        with tc.tile_pool(name="pers", bufs=1) as pers, \
             tc.tile_pool(name="stats", bufs=int(os.environ.get("K_STATS_BUFS", "2"))) as statsp, \
             tc.tile_pool(name="blk", bufs=8, space="PSUM") as ps512, \
             tc.tile_pool(name="wei", bufs=int(os.environ.get("K_WEI_BUFS", "2"))) as weip, \
             tc.tile_pool(name="wpk", bufs=int(os.environ.get("K_WPK_BUFS", "2"))) as wpackp, \
             tc.tile_pool(name="weiT", bufs=int(os.environ.get("K_WEIT_BUFS", "2"))) as weitp, \
             tc.tile_pool(name="osb", bufs=2) as outsp:

            # ---- constants
            # strict-causal additive mask for the diagonal block:
            # diagmask[p, f] = 0 if f < p else NEG_BIG
            diagmask = pers.tile([P, P], f32, tag="diagmask")
            nc.gpsimd.memset(diagmask[:], 0.0)
            nc.gpsimd.affine_select(
                out=diagmask[:], in_=diagmask[:],
                compare_op=mybir.AluOpType.is_gt,
                fill=NEG_BIG, base=0, pattern=[[-1, P]], channel_multiplier=1,
            )
            # -1/WSCALE everywhere except row 0 (zeroes the fully-masked row)
            rowmask0 = pers.tile([P, 1], f32, tag="rowmask0")
            nc.gpsimd.memset(rowmask0[:], -1.0 / WSCALE)
            nc.gpsimd.memset(rowmask0[0:1, :], 0.0)

            # ---- persistent arrays
            x1_sb = pers.tile([P, TB, CCH, 512], fp8, tag="x1", name="x1")
            x2_sb = pers.tile([P, TB, CCH, 512], fp8, tag="x2", name="x2")
            w1_sb = pers.tile([P, CCH, C], fp8, tag="w1", name="w1")
            w2_sb = pers.tile([P, CCH, C], fp8, tag="w2", name="w2")
            # packed fp8 pair of 16*xHT: lo byte = y1, hi byte = y2; [p, j, t]
            ypack = pers.tile([P, CCH, T], u16, tag="ypack", name="ypack")
            y8 = ypack[:].bitcast(fp8).rearrange("p j (t two) -> p j t two", two=2)
            # transposed pack (natural xH layout): [p, t-tile, c]
            zpackT = pers.tile([P, QT, C], u16, tag="zpackT", name="zpackT")
            z8 = zpackT[:].bitcast(fp8).rearrange("p k (c two) -> p k c two", two=2)

            # PE warm-up: dummy bf16 matmuls on scratch (no load deps) fill
            # the initial DMA-wait window and start the HAM clock-gate ramp
            if CFG_WARM:
                warm = pers.tile([P, 512], bf16, tag="warm")
                nc.gpsimd.memset(warm[:], 1.0)
                wps = ps512.tile([P, 512], f32, tag="blk")
                for _k in range(CFG_WARM):
                    nc.tensor.matmul(wps[:], warm[:, 0:P], warm[:],
                                     start=True, stop=True)

            # ---- input loads (sync HWDGE + gpsimd SWDGE rings)
            nc.sync.dma_start(w1_sb[:], w1_d[:, :, :])
            nc.gpsimd.dma_start(w2_sb[:], w2_d[:, :, :])
            for tb in range(TB):
                nc.sync.dma_start(x1_sb[:, tb, :, :], x1_d[:, tb, :, :])
                nc.gpsimd.dma_start(x2_sb[:, tb, :, :], x2_d[:, tb, :, :])

            # ---- phase 1: 16*xHT = (w1+w2)@(x1+x2), 3-term fp8 DoubleRow,
            # quantized straight out of PSUM into the packed pair ypack
            y1eng = nc.scalar if CFG_Y1 == "act" else nc.vector
            for tb in range(TB):
                for dd in range(CCH):
                    pmm = ps512.tile([P, 512], f32, tag="blk")
                    nmm = 0
                    for (wv, xv) in ((w1_sb, x1_sb), (w1_sb, x2_sb),
                                     (w2_sb, x1_sb)):
                        for k in range(DRC):
                            nc.tensor.matmul(
                                pmm[:],
                                wv[:, 2 * k:2 * k + 2, P * dd:P * (dd + 1)],
                                xv[:, tb, 2 * k:2 * k + 2, :],
                                start=(nmm == 0), stop=(nmm == 3 * DRC - 1),
                                perf_mode=DR)
                            nmm += 1
                    ysl1 = y8[:, dd, 512 * tb:512 * (tb + 1), 0]
                    ysl2 = y8[:, dd, 512 * tb:512 * (tb + 1), 1]
                    if CFG_Y1 == "act":
                        nc.scalar.activation(ysl1, pmm[:], Copy)
                    else:
                        nc.vector.tensor_copy(ysl1, pmm[:])
                    nc.vector.tensor_sub(ysl2, pmm[:], ysl1)

            # ---- phase 1b: transpose the packed pair into natural layout
            # (scalar ring: keeps DmaTranspose off the copy-heavy rings;
            # DMATranspose<->DMACopy mixing corrupts data on HW)
            for g in range(2):
                for d in range(CCH):
                    nc.scalar.dma_start_transpose(
                        zpackT[:, 8 * g:8 * g + 8, P * d:P * (d + 1)],
                        ypack[:, d, 1024 * g:1024 * (g + 1)])

            # ---- q-tile loop (software-pipelined emission)
            def emit_scores(i):
                blks = []
                off = 0
                tb = i // 4
                qs = P * (i % 4)
                for n in _sblocks(i):
                    pmm = ps512.tile([P, 512], f32, tag="blk")
                    nmm = 0
                    for (sv, yb) in ((x1_sb, 0), (x1_sb, 1), (x2_sb, 0)):
                        for k in range(DRC):
                            nc.tensor.matmul(
                                pmm[:, :n],
                                sv[:, tb, 2 * k:2 * k + 2, qs:qs + P],
                                y8[:, 2 * k:2 * k + 2, off:off + n, yb],
                                start=(nmm == 0), stop=(nmm == 3 * DRC - 1),
                                perf_mode=DR)
                            nmm += 1
                    blks.append((pmm, off, n))
                    off += n
                # mask the diagonal block (last 128 columns)
                pl, offl, nl = blks[-1]
                nc.vector.tensor_add(pl[:, nl - P:nl], pl[:, nl - P:nl], diagmask[:])
                return blks

            def emit_softmax_out(i, blks):
                nblk = len(blks)
                S = P * (i + 1)
                negblk = statsp.tile([P, 4], f32, tag="negblk")
                for k, (pmm, off, n) in enumerate(blks):
                    nc.vector.reduce_max(negblk[:, k:k + 1], pmm[:, :n],
                                         axis=X, negate=True)
                if nblk > 1:
                    gneg = statsp.tile([P, 1], f32, tag="gneg")
                    nc.vector.tensor_reduce(gneg[:], negblk[:, :nblk],
                                            axis=X, op=mybir.AluOpType.min)
                    gneg_ap = gneg[:]
                else:
                    gneg_ap = negblk[:, 0:1]
                # PSUM holds 16*s: exp((1/16)*psum + (-max(16s))/16)
                gneg_s = statsp.tile([P, 1], f32, tag="gneg_s")
                nc.vector.tensor_scalar_mul(gneg_s[:], gneg_ap, 1.0 / WSCALE)

                sums = statsp.tile([P, 4], f32, tag="sums")
                nc.gpsimd.memset(sums[:, :nblk], 0.0)
                wei = weip.tile([P, S], bf16, tag="wei")
                wpack = wpackp.tile([P, S], u16, tag="wpack")
                wp8 = wpack[:].bitcast(fp8).rearrange("p (s two) -> p s two",
                                                      two=2)
                for k, (pmm, off, n) in enumerate(blks):
                    wsl = wei[:, off:off + n]
                    nc.scalar.activation(wsl, pmm[:, :n], Exp,
                                         bias=gneg_s[:], scale=1.0 / WSCALE,
                                         accum_out=sums[:, k:k + 1])
                    w1sl = wp8[:, off:off + n, 0]
                    if CFG_W1 == "act":
                        nc.scalar.activation(w1sl, wsl, Copy)
                    else:
                        nc.vector.tensor_copy(w1sl, wsl)
                    nc.vector.tensor_sub(wp8[:, off:off + n, 1], wsl, w1sl)
                if nblk > 1:
                    sumtot = statsp.tile([P, 1], f32, tag="sumtot")
                    nc.vector.reduce_sum(sumtot[:], sums[:, :nblk], axis=X)
                    sumtot_ap = sumtot[:]
                else:
                    sumtot_ap = sums[:, 0:1]
                recip = statsp.tile([P, 1], f32, tag="recip")
                nc.vector.reciprocal(recip[:], sumtot_ap)
                if i == 0:
                    nc.vector.tensor_mul(recip[:], recip[:], rowmask0[:])
                else:
                    nc.vector.tensor_scalar_mul(recip[:], recip[:],
                                                -1.0 / WSCALE)

                # transpose packed wei pair via uint16 DMA-xbar (in halves so
                # the first overlaps the remaining exp blocks)
                nk = i + 1
                kpad = nk + (nk & 1)
                wT16 = weitp.tile([P, kpad, P], u16, tag="weiT")
                wT8 = wT16[:].bitcast(fp8).rearrange("p k (q two) -> p k q two",
                                                     two=2)
                if nk & 1:
                    nc.gpsimd.memset(wT16[:, nk:nk + 1, :], 0.0)
                if CFG_SPLIT_WEIT == 2 and nk >= 2:
                    h = nk // 2
                    nc.scalar.dma_start_transpose(
                        wT16[:, :h, :], wpack[:, :P * h])
                    nc.scalar.dma_start_transpose(
                        wT16[:, h:nk, :], wpack[:, P * h:S])
                else:
                    nc.scalar.dma_start_transpose(wT16[:, :nk, :], wpack[:])

                # out = (w1+w2) @ (z1+z2), 3-term fp8 DoubleRow over s-chunks
                osb = outsp.tile([P, C], f32, tag="osb")
                nch = kpad // 2
                for cb in range(CB):
                    opc = ps512.tile([P, 512], f32, tag="blk")
                    nmm = 0
                    for (wb, zb) in ((0, 0), (0, 1), (1, 0)):
                        for j in range(nch):
                            nc.tensor.matmul(
                                opc[:],
                                wT8[:, 2 * j:2 * j + 2, :, wb],
                                z8[:, 2 * j:2 * j + 2,
                                   512 * cb:512 * (cb + 1), zb],
                                start=(nmm == 0), stop=(nmm == 3 * nch - 1),
                                perf_mode=DR)
                            nmm += 1
                    sl = osb[:, 512 * cb:512 * (cb + 1)]
                    use_dve = (CFG_OSB == "dve"
                               or (CFG_OSB == "split" and cb == 1))
                    if use_dve:
                        nc.vector.tensor_scalar_mul(sl, opc[:], recip[:])
                    else:
                        nc.scalar.activation(sl, opc[:], Copy, bias=0.0,
                                             scale=recip[:])
                nc.sync.dma_start(out_d[P * i:P * (i + 1), :], osb[:])

            # q-tile order [1..15, 0]: the tiny tile 0 makes a short tail.
            # Scores emission runs a few tiles ahead so PE always has ready
            # matmuls while a tile's softmax chain drains.
            order = list(range(CFG_TAIL, QT)) + list(range(CFG_TAIL - 1, -1, -1))
            pending = [emit_scores(order[k]) for k in range(CFG_DEPTH)]
            for idx, i in enumerate(order):
                if idx + CFG_DEPTH < QT:
                    pending.append(emit_scores(order[idx + CFG_DEPTH]))
                emit_softmax_out(i, pending.pop(0))

    nc.compile()
    return nc


def _split_fp8(a: np.ndarray):
    a = np.ascontiguousarray(a, dtype=np.float32)
    hi = a.astype(F8)
    lo = (a - hi.astype(np.float32)).astype(F8)
    return hi, lo


def _prep_in_maps(x: np.ndarray, W_H: np.ndarray):
    x = np.asarray(x, dtype=np.float32)
    W_H = np.asarray(W_H, dtype=np.float32)
    wt = W_H.T * WSCALE                      # [c, d] = 16*W[d, c]
    w1, w2 = _split_fp8(wt)
    # [c, d] -> [p, j, d]
    w1 = np.ascontiguousarray(w1.reshape(CCH, P, C).transpose(1, 0, 2))
    w2 = np.ascontiguousarray(w2.reshape(CCH, P, C).transpose(1, 0, 2))
    in_maps = []
    for b in range(B):
        xt = x[b].T                          # [c, t]
        x1, x2 = _split_fp8(xt)
        # [c, t] -> [p, tb, j, u]
        x1 = np.ascontiguousarray(
            x1.reshape(CCH, P, TB, 512).transpose(1, 2, 0, 3))
        x2 = np.ascontiguousarray(
            x2.reshape(CCH, P, TB, 512).transpose(1, 2, 0, 3))
        in_maps.append({"x1": x1, "x2": x2, "w1": w1, "w2": w2})
    return in_maps


def kernel(x: np.ndarray, W_H: np.ndarray) -> np.ndarray:
    from concourse import bass_utils

    nc = _build()
    in_maps = _prep_in_maps(x, W_H)
    res = bass_utils.run_bass_kernel_spmd(nc, in_maps, core_ids=list(range(NCORES)))
    return np.stack([res.results[b]["out"] for b in range(B)])


if __name__ == "__main__":
    x = np.random.randn(B, T, C).astype(np.float32)
    W = (np.random.randn(C, C) / np.sqrt(C)).astype(np.float32)
    out = kernel(x, W)
    print("out", out.shape, out.dtype)


# revision 85
# speedup vs baseline: 1.5390x; 1.5390x over previous
"""EnergyHead kernel for Trainium2 (8 NeuronCores, batch-parallel, fp8 DoubleRow).

Computes, per batch element:
    xH = x @ W_H.T
    scores = x @ xH.T  (strict lower-triangular causal mask, diag excluded)
    wei = softmax(scores); fully-masked row 0 zeroed
    out = -(wei @ xH)

Sharding: data-parallel over B=8 across 8 cores. All three matmul phases run
in fp8e4m3 DoubleRow mode (K=256 per instruction, 0.5 cycles/row) with
residual-split operands for precision: every tensor a is represented as
a1 + a2 where a1 = e4m3(a), a2 = e4m3(a - a1), and products use the 3-term
expansion a1b1 + a1b2 + a2b1 (the dropped a2b2 term is O(2^-8) relative).
W is pre-scaled by 32 on the host so all fp8 magnitudes stay well under the
e4m3 max (240) while keeping the residuals out of subnormal range; the 1/32
is folded into the softmax bias/scale and the final recip multiply.
End-to-end rel err vs fp32 reference ~1e-2 (gate: 2e-2).

Schedule: per-tile softmax chains (mask -> blockwise max -> exp+accum ->
fp8 pack -> uint16 DMA transpose -> out-matmul) are pipelined 4-5 tiles
deep against PE score matmuls (deeper for small early tiles); the first
four q-tiles are interleaved into phase 1's t-block stream; osb scales are
deferred one tile so ACT/DVE in-order queues never block the next tile's
exp chain; DMA transposes are issued from the otherwise-idle sync queue
(gpsimd engines cannot access PSUM, and a DmaTranspose costs ~660ns on
the issuing sequencer); wpack/weiT pools run 3 buffers deep to decouple
consecutive tiles' pack/transpose stages.
"""
import sys
import os
import functools

sys.path.insert(0, "/opt/trn_rl_repo")
import numpy as np
import ml_dtypes

# experiment flags (read once at import)
CFG_SPLIT_WEIT = int(os.environ.get("K_SPLIT_WEIT", "1"))  # 1 | 2
CFG_DEPTH = int(os.environ.get("K_DEPTH", "5"))    # scores prefetch depth
CFG_WARM = int(os.environ.get("K_WARM", "12"))
CFG_TAIL = int(os.environ.get("K_TAIL", "1"))
CFG_Y1 = os.environ.get("K_Y1", "act")             # act | dve
CFG_W1 = os.environ.get("K_W1", "act")             # act | dve
CFG_OSB = os.environ.get("K_OSB", "split")         # act | dve | split
CFG_MASK = os.environ.get("K_MASK", "dve")         # pool | dve
CFG_PRO = int(os.environ.get("K_PRO", "3"))        # prologue tile count
CFG_WEIT_RING = os.environ.get("K_WEIT_RING", "sync")  # act | sync
CFG_STORE_RING = os.environ.get("K_STORE_RING", "sync")  # sync | pool

B, T, C = 8, 2048, 1024
NCORES = 8
P = 128                      # partition dim
QT = T // P                  # 16 q-tiles
CCH = C // P                 # 8 contraction 128-chunks
DRC = C // 256               # 4 DoubleRow 256-chunks
TB = T // 512                # 4 t-blocks
CB = C // 512                # 2 column blocks of the output
NEG_BIG = -1e30
WSCALE = 32.0
F8 = ml_dtypes.float8_e4m3


def _sblocks(i: int) -> list[int]:
    """Split S=128*(i+1) score columns into <=512 matmul N-blocks."""
    S = P * (i + 1)
    k4, rem = divmod(S, 512)
    return [512] * k4 + ([rem] if rem else [])


@functools.lru_cache(maxsize=4)
def _build(reps: int = 1):
    import concourse.bacc as bacc
    import concourse.tile as tile
    from concourse import mybir

    f32 = mybir.dt.float32
    fp8 = mybir.dt.float8e4
    u16 = mybir.dt.uint16
    bf16 = mybir.dt.bfloat16
    X = mybir.AxisListType.X
    Exp = mybir.ActivationFunctionType.Exp
    Copy = mybir.ActivationFunctionType.Copy
    DR = mybir.MatmulPerfMode.DoubleRow

    nc = bacc.Bacc("TRN2", target_bir_lowering=False, debug=False,
                   enable_asserts=False, num_devices=NCORES)

    # host layouts:
    #   x1/x2: [p, tb, j, u] = x.T[128j+p, 512tb+u]  (fp8 value + residual)
    #   w1/w2: [p, j, d]     = 16*W.T[128j+p, d]
    x1_d = nc.dram_tensor("x1", [P, TB, CCH, 512], fp8, kind="ExternalInput").ap()
    x2_d = nc.dram_tensor("x2", [P, TB, CCH, 512], fp8, kind="ExternalInput").ap()
    w1_d = nc.dram_tensor("w1", [P, CCH, C], fp8, kind="ExternalInput").ap()
    w2_d = nc.dram_tensor("w2", [P, CCH, C], fp8, kind="ExternalInput").ap()
    out_d = nc.dram_tensor("out", [T, C], f32, kind="ExternalOutput").ap()

    with tile.TileContext(nc) as tc:
      for _rep in range(reps):
        with tc.tile_pool(name="pers", bufs=1) as pers, \
             tc.tile_pool(name="stats", bufs=int(os.environ.get("K_STATS_BUFS", str(2 + int(os.environ.get("K_OUTLAG", "0"))))) ) as statsp, \
             tc.tile_pool(name="blk", bufs=8, space="PSUM") as ps512, \
             tc.tile_pool(name="wei", bufs=int(os.environ.get("K_WEI_BUFS", "2"))) as weip, \
             tc.tile_pool(name="wpk", bufs=int(os.environ.get("K_WPK_BUFS", "3"))) as wpackp, \
             tc.tile_pool(name="weiT", bufs=int(os.environ.get("K_WEIT_BUFS", "3"))) as weitp, \
             tc.tile_pool(name="osb", bufs=2) as outsp:

            # ---- constants
            # strict-causal additive mask for the diagonal block:
            # diagmask[p, f] = 0 if f < p else NEG_BIG
            diagmask = pers.tile([P, P], f32, tag="diagmask")
            nc.gpsimd.memset(diagmask[:], 0.0)
            nc.gpsimd.affine_select(
                out=diagmask[:], in_=diagmask[:],
                compare_op=mybir.AluOpType.is_gt,
                fill=NEG_BIG, base=0, pattern=[[-1, P]], channel_multiplier=1,
            )
            # -1/WSCALE everywhere except row 0 (zeroes the fully-masked row)
            rowmask0 = pers.tile([P, 1], f32, tag="rowmask0")
            nc.gpsimd.memset(rowmask0[:], -1.0 / WSCALE)
            nc.gpsimd.memset(rowmask0[0:1, :], 0.0)

            # ---- persistent arrays
            x1_sb = pers.tile([P, TB, CCH, 512], fp8, tag="x1", name="x1")
            x2_sb = pers.tile([P, TB, CCH, 512], fp8, tag="x2", name="x2")
            w1_sb = pers.tile([P, CCH, C], fp8, tag="w1", name="w1")
            w2_sb = pers.tile([P, CCH, C], fp8, tag="w2", name="w2")
            # packed fp8 pair of 16*xHT: lo byte = y1, hi byte = y2; [p, j, t]
            ypack = pers.tile([P, CCH, T], u16, tag="ypack", name="ypack")
            y8 = ypack[:].bitcast(fp8).rearrange("p j (t two) -> p j t two", two=2)
            # transposed pack (natural xH layout): [p, t-tile, c]
            zpackT = pers.tile([P, QT, C], u16, tag="zpackT", name="zpackT")
            z8 = zpackT[:].bitcast(fp8).rearrange("p k (c two) -> p k c two", two=2)

            # PE warm-up: dummy bf16 matmuls on scratch (no load deps) fill
            # the initial DMA-wait window and start the HAM clock-gate ramp
            if CFG_WARM:
                warm = pers.tile([P, 512], bf16, tag="warm")
                nc.gpsimd.memset(warm[:], 1.0)
                wps = ps512.tile([P, 512], f32, tag="blk")
                for _k in range(CFG_WARM):
                    nc.tensor.matmul(wps[:], warm[:, 0:P], warm[:],
                                     start=True, stop=True)

            # ---- input loads.  The DMA engine pool serializes transfers and
            # drains the sync (HWDGE) queue ahead of the gpsimd ring, so the
            # transfers are issued in exact first-consumption order: the
            # (tb0, dd0-1) group reads w*[d 0:256] + x*[tb0], with terms
            # ordered (w1x1, w2x1, w1x2) to match arrival.  The sync queue
            # later carries the DMA transposes (never concurrent with these).
            if int(os.environ.get("K_FINELOAD", "0")):
                nc.sync.dma_start(w1_sb[:, :, 0:128], w1_d[:, :, 0:128])
                nc.sync.dma_start(x1_sb[:, 0, :, :], x1_d[:, 0, :, :])
                nc.sync.dma_start(w2_sb[:, :, 0:128], w2_d[:, :, 0:128])
                nc.sync.dma_start(x2_sb[:, 0, :, :], x2_d[:, 0, :, :])
                nc.sync.dma_start(w1_sb[:, :, 128:512], w1_d[:, :, 128:512])
                nc.sync.dma_start(w2_sb[:, :, 128:512], w2_d[:, :, 128:512])
                nc.gpsimd.dma_start(w1_sb[:, :, 512:1024],
                                    w1_d[:, :, 512:1024])
            else:
                nc.sync.dma_start(w1_sb[:, :, 0:512], w1_d[:, :, 0:512])
                nc.sync.dma_start(x1_sb[:, 0, :, :], x1_d[:, 0, :, :])
                nc.sync.dma_start(w1_sb[:, :, 512:1024], w1_d[:, :, 512:1024])
                nc.sync.dma_start(w2_sb[:, :, 0:512], w2_d[:, :, 0:512])
                nc.sync.dma_start(x2_sb[:, 0, :, :], x2_d[:, 0, :, :])
            nc.gpsimd.dma_start(w2_sb[:, :, 512:1024], w2_d[:, :, 512:1024])
            for tb in range(1, TB):
                nc.gpsimd.dma_start(x1_sb[:, tb, :, :], x1_d[:, tb, :, :])
                nc.gpsimd.dma_start(x2_sb[:, tb, :, :], x2_d[:, tb, :, :])

            # ---- phase 1: 16*xHT = (w1+w2)@(x1+x2), 3-term fp8 DoubleRow,
            # quantized straight out of PSUM into the packed pair ypack.
            # tb0 runs term-major ("sweeps"): all 8 dd-groups open at once
            # (exactly the 8 PSUM banks), so the w1x1 sweep starts as soon as
            # w1+x1[tb0] land instead of waiting for all four tensors.
            def _p1_quant(tb, dd, pmm):
                ysl1 = y8[:, dd, 512 * tb:512 * (tb + 1), 0]
                ysl2 = y8[:, dd, 512 * tb:512 * (tb + 1), 1]
                if CFG_Y1 == "act":
                    nc.scalar.activation(ysl1, pmm[:], Copy)
                else:
                    nc.vector.tensor_copy(ysl1, pmm[:])
                nc.vector.tensor_sub(ysl2, pmm[:], ysl1)

            P1_TERMS = ((w1_sb, x1_sb), (w2_sb, x1_sb), (w1_sb, x2_sb))

            def emit_phase1_tb(tb):
                if tb == 0 and int(os.environ.get("K_SWEEP0", "0")):
                    pmms = [ps512.tile([P, 512], f32, tag="blk",
                                       name=f"p1sw{dd}")
                            for dd in range(CCH)]
                    for ti, (wv, xv) in enumerate(P1_TERMS):
                        for dd in range(CCH):
                            for k in range(DRC):
                                nc.tensor.matmul(
                                    pmms[dd][:],
                                    wv[:, 2 * k:2 * k + 2,
                                       P * dd:P * (dd + 1)],
                                    xv[:, tb, 2 * k:2 * k + 2, :],
                                    start=(ti == 0 and k == 0),
                                    stop=(ti == 2 and k == DRC - 1),
                                    perf_mode=DR)
                        if ti == 2:
                            for dd in range(CCH):
                                _p1_quant(tb, dd, pmms[dd])
                else:
                    for dd in range(CCH):
                        pmm = ps512.tile([P, 512], f32, tag="blk")
                        nmm = 0
                        for (wv, xv) in P1_TERMS:
                            for k in range(DRC):
                                nc.tensor.matmul(
                                    pmm[:],
                                    wv[:, 2 * k:2 * k + 2, P * dd:P * (dd + 1)],
                                    xv[:, tb, 2 * k:2 * k + 2, :],
                                    start=(nmm == 0), stop=(nmm == 3 * DRC - 1),
                                    perf_mode=DR)
                                nmm += 1
                        _p1_quant(tb, dd, pmm)
                # phase 1b interleaved: transpose this t-block of the packed
                # pair into natural layout as soon as its quantization lands,
                # overlapping the remaining phase-1 compute.  Issued from the
                # otherwise idle sync queue (a DmaTranspose's ~650ns issue
                # cost would serialize against compute on a busy sequencer;
                # the sync ring also never mixes with in-flight DMACopies).
                for d in range(CCH):
                    nc.sync.dma_start_transpose(
                        zpackT[:, 4 * tb:4 * (tb + 1), P * d:P * (d + 1)],
                        ypack[:, d, 512 * tb:512 * (tb + 1)])

            # ---- q-tile emitters (software-pipelined emission)
            def emit_scores(i):
                blks = []
                off = 0
                tb = i // 4
                qs = P * (i % 4)
                for n in _sblocks(i):
                    pmm = ps512.tile([P, 512], f32, tag="blk")
                    nmm = 0
                    for (sv, yb) in ((x1_sb, 0), (x1_sb, 1), (x2_sb, 0)):
                        for k in range(DRC):
                            nc.tensor.matmul(
                                pmm[:, :n],
                                sv[:, tb, 2 * k:2 * k + 2, qs:qs + P],
                                y8[:, 2 * k:2 * k + 2, off:off + n, yb],
                                start=(nmm == 0), stop=(nmm == 3 * DRC - 1),
                                perf_mode=DR)
                            nmm += 1
                    blks.append((pmm, off, n))
                    off += n
                return blks

            def emit_softmax_out(i, blks):
                nblk = len(blks)
                S = P * (i + 1)
                # mask the diagonal block (last 128 cols).  Emitted here, not
                # in emit_scores: a mask op queued at prefetch time would sit
                # in its engine's in-order stream ahead of this tile's
                # reduce/exp chain while waiting on far-future matmuls.
                pl, offl, nl = blks[-1]
                nc.vector.tensor_add(pl[:, nl - P:nl], pl[:, nl - P:nl], diagmask[:])
                negblk = statsp.tile([P, 4], f32, tag="negblk")
                for k, (pmm, off, n) in enumerate(blks):
                    nc.vector.reduce_max(negblk[:, k:k + 1], pmm[:, :n],
                                         axis=X, negate=True)
                if nblk > 1:
                    gneg = statsp.tile([P, 1], f32, tag="gneg")
                    nc.vector.tensor_reduce(gneg[:], negblk[:, :nblk],
                                            axis=X, op=mybir.AluOpType.min)
                    gneg_ap = gneg[:]
                else:
                    gneg_ap = negblk[:, 0:1]
                # PSUM holds 16*s: exp((1/16)*psum + (-max(16s))/16)
                gneg_s = statsp.tile([P, 1], f32, tag="gneg_s")
                nc.vector.tensor_scalar_mul(gneg_s[:], gneg_ap, 1.0 / WSCALE)

                sums = statsp.tile([P, 4], f32, tag="sums")
                nc.gpsimd.memset(sums[:, :nblk], 0.0)
                wei = weip.tile([P, S], bf16, tag="wei")
                wpack = wpackp.tile([P, S], u16, tag="wpack")
                wp8 = wpack[:].bitcast(fp8).rearrange("p (s two) -> p s two",
                                                      two=2)

                # transpose the packed wei pair via uint16 DMA-xbar, split at
                # a 512-block boundary so the first half is issued as soon as
                # its packs land (overlapping the remaining exp blocks)
                nk = i + 1
                kpad = nk + (nk & 1)
                wT16 = weitp.tile([P, kpad, P], u16, tag="weiT")
                wT8 = wT16[:].bitcast(fp8).rearrange("p k (q two) -> p k q two",
                                                     two=2)
                if nk & 1:
                    nc.gpsimd.memset(wT16[:, nk:nk + 1, :], 0.0)
                teng = nc.scalar if CFG_WEIT_RING == "act" else nc.sync
                hb = (nblk + 1) // 2
                split_blk = hb - 1 if (CFG_SPLIT_WEIT == 2
                                       and nblk >= 2) else None
                h = 4 * hb if split_blk is not None else None

                for k, (pmm, off, n) in enumerate(blks):
                    wsl = wei[:, off:off + n]
                    nc.scalar.activation(wsl, pmm[:, :n], Exp,
                                         bias=gneg_s[:], scale=1.0 / WSCALE,
                                         accum_out=sums[:, k:k + 1])
                    w1sl = wp8[:, off:off + n, 0]
                    if CFG_W1 == "act":
                        nc.scalar.activation(w1sl, wsl, Copy)
                    elif CFG_W1 == "pool":
                        nc.gpsimd.tensor_copy(w1sl, wsl)
                    else:
                        nc.vector.tensor_copy(w1sl, wsl)
                    nc.vector.tensor_sub(wp8[:, off:off + n, 1], wsl, w1sl)
                    if split_blk is not None and k == split_blk:
                        teng.dma_start_transpose(wT16[:, :h, :],
                                                 wpack[:, :P * h])
                if split_blk is not None:
                    teng.dma_start_transpose(wT16[:, h:nk, :],
                                             wpack[:, P * h:S])
                else:
                    teng.dma_start_transpose(wT16[:, :nk, :], wpack[:])
                if nblk > 1:
                    sumtot = statsp.tile([P, 1], f32, tag="sumtot")
                    nc.vector.reduce_sum(sumtot[:], sums[:, :nblk], axis=X)
                    sumtot_ap = sumtot[:]
                else:
                    sumtot_ap = sums[:, 0:1]
                recip = statsp.tile([P, 1], f32, tag="recip")
                nc.vector.reciprocal(recip[:], sumtot_ap)
                if i == 0:
                    nc.vector.tensor_mul(recip[:], recip[:], rowmask0[:])
                else:
                    nc.vector.tensor_scalar_mul(recip[:], recip[:],
                                                -1.0 / WSCALE)

                return (i, wT8, kpad, recip)

            def emit_out(state, last=False):
                i, wT8, kpad, recip = state
                # out = (w1+w2) @ (z1+z2), 3-term fp8 DoubleRow over s-chunks.
                # For the final tile the second half runs as two 256-col
                # groups so its scale/store pipeline drains a shorter tail.
                nch = kpad // 2
                segs = [(0, 512), (512, 512)] if not last else \
                       [(0, 512), (512, 256), (768, 128), (896, 128)]
                osegs = []
                for (co, cn) in segs:
                    opc = ps512.tile([P, cn], f32, tag="blk", name=f"opc{co}")
                    nmm = 0
                    for (wb, zb) in ((0, 0), (0, 1), (1, 0)):
                        for j in range(nch):
                            nc.tensor.matmul(
                                opc[:],
                                wT8[:, 2 * j:2 * j + 2, :, wb],
                                z8[:, 2 * j:2 * j + 2, co:co + cn, zb],
                                start=(nmm == 0), stop=(nmm == 3 * nch - 1),
                                perf_mode=DR)
                            nmm += 1
                    osegs.append((co, cn, opc))
                return (i, osegs, recip)

            def emit_osb_store(state, last=False):
                # Emitted one tile AFTER the out-matmuls (gpsimd cannot read
                # PSUM, so the scales must run on ACT/DVE — deferring them
                # keeps those engines' in-order streams from blocking the
                # next tile's exp chain on this tile's out-matmuls).
                i, osegs, recip = state
                osb = outsp.tile([P, C], f32, tag="osb")
                seng = nc.sync if CFG_STORE_RING == "sync" else nc.gpsimd
                for si, (co, cn, opc) in enumerate(osegs):
                    sl = osb[:, co:co + cn]
                    if CFG_OSB == "dve" or (CFG_OSB in ("split", "pool")
                                            and si % 2 == 1):
                        nc.vector.tensor_scalar_mul(sl, opc[:], recip[:])
                    else:
                        nc.scalar.activation(sl, opc[:], Copy, bias=0.0,
                                             scale=recip[:])
                    if last:
                        # final tile only: per-segment store so each DMA
                        # overlaps the next segment's scale (shorter tail)
                        seng.dma_start(out_d[P * i:P * (i + 1), co:co + cn],
                                       sl)
                if not last:
                    seng.dma_start(out_d[P * i:P * (i + 1), :], osb[:])

            # q-tile order [1..15, 0]: the tiny tile 0 makes a short tail.
            # Scores emission runs a few tiles ahead so PE always has ready
            # matmuls while a tile's softmax chain drains.
            # Softmax+out of tile i is emitted BEFORE topping up prefetched
            # scores: the out-matmuls must not sit behind future score
            # matmuls in PE's in-order stream (those may stall on PSUM banks
            # that only the out path frees).  Prefetch depth is budgeted in
            # 512-col psum blocks, so small early tiles prefetch deeper.
            # ---- main schedule.  The first q-tiles are interleaved into the
            # phase-1 t-block stream: their scores only need tb0's y-chunks,
            # so their softmax chains run on DVE/ACT while the PE grinds
            # through the later phase-1 blocks, and their outputs are done
            # before the dense q-loop starts.  Tile 0 (tiny, fully-masked
            # row) is scheduled mid-stream so the kernel tail ends on a big
            # tile whose long out-matmul hides its own softmax chain.
            t0pos = int(os.environ.get("K_T0POS", "3"))
            if int(os.environ.get("K_DESC", "0")):
                order = [1, 2, 3, 0] + list(range(QT - 1, 3, -1))
            else:
                order = list(range(1, QT))
                order.insert(t0pos, 0) if 0 <= t0pos <= len(order) else order.append(0)
            pending = []        # [(tile, blks)]
            outq = []           # deferred out states (see K_OUTLAG)
            osbq = []           # deferred osb-scale/store states
            outlag = int(os.environ.get("K_OUTLAG", "0"))
            osblag = int(os.environ.get("K_OSBLAG", "1"))
            nxt = 0

            def emit_next_scores():
                nonlocal nxt
                if nxt < len(order):
                    pending.append((order[nxt], emit_scores(order[nxt])))
                    nxt += 1

            def emit_next_softmax(final=False):
                # Deferring each tile's out-matmuls by K_OUTLAG tiles gives
                # the softmax->pack->transpose chain that much slack before
                # PE reaches the out, and banks PE work for the endgame when
                # no score tiles remain.
                i, blks = pending.pop(0)
                outq.append(emit_softmax_out(i, blks))
                while len(outq) > outlag:
                    st = outq.pop(0)
                    osbq.append(emit_out(st, last=(final and not outq)))
                while len(osbq) > osblag:
                    emit_osb_store(osbq.pop(0))

            inter = int(os.environ.get("K_INTER", "2"))
            late_depth = int(os.environ.get("K_LATE_DEPTH", str(CFG_DEPTH)))
            late_from = int(os.environ.get("K_LATE_FROM", "12"))
            if inter == 2:
                # tiles 1,2,3,0 fully pipelined inside the phase-1 stream
                emit_phase1_tb(0)
                for _ in range(4):
                    emit_next_scores()          # tiles in tb0 only
                emit_phase1_tb(1)
                emit_next_softmax()
                emit_phase1_tb(2)
                emit_next_softmax()
                emit_next_scores()
                emit_phase1_tb(3)
                emit_next_softmax()
                emit_next_softmax()
                for _ in range(2):
                    emit_next_scores()
            elif inter == 1:
                emit_phase1_tb(0)
                for _ in range(3):
                    emit_next_scores()          # tiles in tb0 only
                emit_phase1_tb(1)
                emit_next_softmax()
                emit_phase1_tb(2)
                emit_next_softmax()
                emit_next_scores()
                emit_phase1_tb(3)
                emit_next_softmax()
                for _ in range(2):
                    emit_next_scores()
            else:
                for tb in range(TB):
                    emit_phase1_tb(tb)
                for _ in range(CFG_DEPTH):
                    emit_next_scores()
            early_depth = int(os.environ.get("K_DEPTH_EARLY", "5"))
            while pending:
                emit_next_softmax(final=(len(pending) == 1
                                         and nxt >= len(order)))
                if nxt >= late_from:
                    depth = late_depth
                elif nxt < len(order) and order[nxt] <= 7:
                    # small tiles (<=2 score blocks) prefetch deeper: their
                    # per-tile PE work is shorter than the softmax chain
                    depth = early_depth
                else:
                    depth = CFG_DEPTH
                while len(pending) < depth and nxt < len(order):
                    emit_next_scores()
            while outq:
                st = outq.pop(0)
                osbq.append(emit_out(st, last=(not outq)))
            while osbq:
                st = osbq.pop(0)
                emit_osb_store(st, last=(not osbq))

    nc.compile()
    return nc


def _split_fp8(a: np.ndarray):
    a = np.ascontiguousarray(a, dtype=np.float32)
    hi = a.astype(F8)
    lo = (a - hi.astype(np.float32)).astype(F8)
    return hi, lo


def _prep_in_maps(x: np.ndarray, W_H: np.ndarray):
    x = np.asarray(x, dtype=np.float32)
    W_H = np.asarray(W_H, dtype=np.float32)
    wt = W_H.T * WSCALE                      # [c, d] = 16*W[d, c]
    w1, w2 = _split_fp8(wt)
    # [c, d] -> [p, j, d]
    w1 = np.ascontiguousarray(w1.reshape(CCH, P, C).transpose(1, 0, 2))
    w2 = np.ascontiguousarray(w2.reshape(CCH, P, C).transpose(1, 0, 2))
    in_maps = []
    for b in range(B):
        xt = x[b].T                          # [c, t]
        x1, x2 = _split_fp8(xt)
        # [c, t] -> [p, tb, j, u]
        x1 = np.ascontiguousarray(
            x1.reshape(CCH, P, TB, 512).transpose(1, 2, 0, 3))
        x2 = np.ascontiguousarray(
            x2.reshape(CCH, P, TB, 512).transpose(1, 2, 0, 3))
        in_maps.append({"x1": x1, "x2": x2, "w1": w1, "w2": w2})
    return in_maps


def kernel(x: np.ndarray, W_H: np.ndarray) -> np.ndarray:
    from concourse import bass_utils

    nc = _build()
    in_maps = _prep_in_maps(x, W_H)
    res = bass_utils.run_bass_kernel_spmd(nc, in_maps, core_ids=list(range(NCORES)))
    return np.stack([res.results[b]["out"] for b in range(B)])


if __name__ == "__main__":
    x = np.random.randn(B, T, C).astype(np.float32)
    W = (np.random.randn(C, C) / np.sqrt(C)).astype(np.float32)
    out = kernel(x, W)
    print("out", out.shape, out.dtype)
